# revision 27
# baseline (speedup 1.0000x reference)
"""Trainium2 Bass kernel for the GAT+CNN+MLP pair-scoring model.

Strategy (8 NeuronCores, one SPMD NEFF, one AllGather):
  - node projections (h, xl, xr) replicated on every core (cheap),
  - GAT edge softmax + aggregation + CNN split across cores by
    destination-node blocks (edges pre-sorted by dst on host),
  - cnn embeddings all-gathered in bf16,
  - pair gather + MLP split across cores by pair index.
"""

import sys

sys.path.insert(0, "/opt/trn_rl_repo")

import ml_dtypes
import numpy as np

import concourse.bass as bass
import concourse.mybir as mybir
import concourse.tile as tile
from concourse import bacc
from concourse.bass import IndirectOffsetOnAxis
from concourse.bass_utils import run_bass_kernel_spmd
from concourse.masks import make_identity

F32 = mybir.dt.float32
BF16 = mybir.dt.bfloat16
I32 = mybir.dt.int32
AF = mybir.ActivationFunctionType
OP = mybir.AluOpType

N_CORES = 8
NM, ND = 1500, 800
NN = NM + ND            # 2300 nodes
NPAD = 2304             # padded node count (24 blocks of 96)
BLK = 96                # dst-node block size
NBLK = NPAD // BLK      # 24
BPC = NBLK // N_CORES   # 3 blocks per core
C, H = 128, 8
HC = H * C              # 1024
WREAL = 2778            # true cnn_outputs width
WE = 2816               # padded emb width (22 k-tiles of 128)
L1, L2, L3 = 1389, 694, 463
GAT_SLOPE = 0.2
MLP_SLOPE = 0.01
NPAIR = 80000
PPC = NPAIR // N_CORES  # 10000 pairs per core
PT = 512                # pair tile
NPT = 10240 // PT       # 20 pair tiles per core (10240 = padded)
PPAD = NPT * PT


def _tiles(total, step=128):
    return [(i, min(step, total - i)) for i in range(0, total, step)]


def _f32(x):
    return np.ascontiguousarray(np.asarray(x), dtype=np.float32)


def _bf(x):
    return np.ascontiguousarray(np.asarray(x)).astype(ml_dtypes.bfloat16)


def _prep_host(inputs):
    """All host-side data preparation: casts, layout transforms, index prep."""
    d = {}
    mic = _f32(inputs["mic_feature"])
    dis = _f32(inputs["dis_feature"])
    d["micT"] = np.ascontiguousarray(mic.T)
    d["disT"] = np.ascontiguousarray(dis.T)
    d["W_mic"] = _f32(inputs["W_mic"])
    d["W_dis"] = _f32(inputs["W_dis"])
    d["Wl"] = _f32(inputs["gat_Wl"])
    d["Wr"] = _f32(inputs["gat_Wr"])

    bl = _f32(inputs["gat_bl"]).reshape(-1)
    br = _f32(inputs["gat_br"]).reshape(-1)
    d["has_bl"] = bool(np.any(bl)) or bool(np.any(br))
    d["bl128"] = np.ascontiguousarray(np.broadcast_to(bl, (128, HC)))
    d["br128"] = np.ascontiguousarray(np.broadcast_to(br, (128, HC)))

    att = _f32(inputs["gat_att"]).reshape(-1)  # [1024] flat (h,c)
    d["att128"] = np.ascontiguousarray(np.broadcast_to(att, (128, HC)))

    gb = _f32(inputs["gat_bias"]).reshape(-1)  # [1024]
    d["has_gb"] = bool(np.any(gb))
    d["gbT"] = np.ascontiguousarray(gb.reshape(H, C).T)  # [128, 8]

    # CNN as one dense [1024, WE] matmul (+ bias row)
    Wc = np.zeros((H, C, WE), np.float32)
    bc = np.zeros(WE, np.float32)
    col = 0
    for wname, bname in (("cw1", "cb1"), ("cw4", "cb4"),
                         ("cw16", "cb16"), ("cw32", "cb32")):
        w = _f32(inputs[wname])  # [6, 1, 8, k]
        b = _f32(inputs[bname])  # [6]
        O, _, _, K = w.shape
        J = C - K + 1
        j = np.arange(J)
        for o in range(O):
            cols = col + o * J + j
            for dk in range(K):
                Wc[:, dk + j, cols] = w[o, 0, :, dk][:, None]
            bc[cols] = b[o]
        col += O * J
    assert col == WREAL
    d["Wcnn"] = _bf(Wc.reshape(HC, WE))
    d["has_bc"] = bool(np.any(bc))
    d["bcnn128"] = np.ascontiguousarray(np.broadcast_to(bc, (128, WE)))

    # MLP weights (pad mw1 rows to WE with zeros)
    mw1 = np.zeros((WE, L1), np.float32)
    mw1[:WREAL] = _f32(inputs["mw1"])
    d["mw1"] = _bf(mw1)
    d["mw2"] = _bf(_f32(inputs["mw2"]))
    d["mw3"] = _bf(_f32(inputs["mw3"]))
    mw4 = _f32(inputs["mw4"]).reshape(L3, 1)
    d["mw4"] = _bf(mw4)
    mb1 = _f32(inputs["mb1"]).reshape(-1, 1)
    mb2 = _f32(inputs["mb2"]).reshape(-1, 1)
    mb3 = _f32(inputs["mb3"]).reshape(-1, 1)
    d["has_mb"] = bool(np.any(mb1)) or bool(np.any(mb2)) or bool(np.any(mb3))
    d["mb1"], d["mb2"], d["mb3"] = mb1, mb2, mb3

    # ---- edges: append self loops, sort by dst, partition into blocks ----
    ei = np.asarray(inputs["edge_index"])
    src = np.concatenate([ei[0], np.arange(NN)]).astype(np.int64)
    dst = np.concatenate([ei[1], np.arange(NN)]).astype(np.int64)
    order = np.argsort(dst, kind="stable")
    src = src[order].astype(np.int32)
    dst = dst[order].astype(np.int32)
    blk_of = dst // BLK
    counts = np.bincount(blk_of, minlength=NBLK)
    T = int(np.ceil(counts.max() / 128))
    cap = T * 128
    eidx = np.zeros((NBLK, cap, 2), np.int32)
    msel = np.zeros((NBLK, cap, BLK), np.float32)
    pos = 0
    for b in range(NBLK):
        n = int(counts[b])
        s = src[pos:pos + n]
        dd = dst[pos:pos + n]
        pos += n
        eidx[b, :n, 0] = s
        eidx[b, :n, 1] = dd
        eidx[b, n:, 0] = 0
        eidx[b, n:, 1] = min(b * BLK, NN - 1)
        msel[b, np.arange(n), dd - b * BLK] = 1.0
    assert pos == len(src)
    d["T"] = T
    # per-core, partition-major layouts
    eidx = eidx.reshape(NBLK, T, 128, 2)
    msel = msel.reshape(NBLK, T, 128, BLK)
    d["eidx_pc"] = []
    d["msel_pc"] = []
    for c in range(N_CORES):
        e_c = eidx[c * BPC:(c + 1) * BPC].reshape(BPC * T, 128, 2)
        m_c = msel[c * BPC:(c + 1) * BPC].reshape(BPC * T, 128, BLK)
        d["eidx_pc"].append(np.ascontiguousarray(e_c.transpose(1, 0, 2)))
        d["msel_pc"].append(_bf(m_c.transpose(1, 0, 2)))

    # ---- pairs ----
    pos_p = np.asarray(inputs["pos_pairs"])
    neg_p = np.asarray(inputs["neg_pairs"])
    pairs = np.concatenate([pos_p, neg_p], axis=0).astype(np.int64)
    gidx = np.zeros((NPAIR, 2), np.int32)
    gidx[:, 0] = pairs[:, 0]
    gidx[:, 1] = NM + pairs[:, 1]
    d["pidx_pc"] = []
    for c in range(N_CORES):
        p_c = np.zeros((PPAD, 2), np.int32)
        p_c[:, 1] = NM  # pad pairs -> (0, NM)
        p_c[:PPC] = gidx[c * PPC:(c + 1) * PPC]
        # [128, NPT, 4, 2] partition-major
        p_c = p_c.reshape(NPT, 4, 128, 2).transpose(2, 0, 1, 3)
        d["pidx_pc"].append(np.ascontiguousarray(p_c))

    d["labels"] = np.concatenate(
        [np.ones((PPC * 4, 1), np.float32), np.zeros((PPC * 4, 1), np.float32)]
    )
    return d


def _build_program(T, has_bl, has_gb, has_bc, has_mb, debug=False):
    """Emit the full SPMD program. Returns (nc, names of I/O tensors)."""
    nc = bacc.Bacc("TRN2", target_bir_lowering=False, debug=False,
                   num_devices=N_CORES)

    # ---------------- DRAM I/O ----------------
    def din(name, shape, dt):
        return nc.dram_tensor(name, shape, dt, kind="ExternalInput")

    micT_d = din("micT", [NM, NM], F32)
    disT_d = din("disT", [ND, ND], F32)
    wmic_d = din("W_mic", [NM, C], F32)
    wdis_d = din("W_dis", [ND, C], F32)
    wl_d = din("Wl", [C, HC], F32)
    wr_d = din("Wr", [C, HC], F32)
    att_d = din("att128", [128, HC], F32)
    wcnn_d = din("Wcnn", [HC, WE], BF16)
    mw1_d = din("mw1", [WE, L1], BF16)
    mw2_d = din("mw2", [L1, L2], BF16)
    mw3_d = din("mw3", [L2, L3], BF16)
    mw4_d = din("mw4", [L3, 1], BF16)
    eidx_d = din("eidx", [128, BPC * T, 2], I32)
    msel_d = din("msel", [128, BPC * T, BLK], BF16)
    pidx_d = din("pidx", [128, NPT, 4, 2], I32)
    bl_d = din("bl128", [128, HC], F32) if has_bl else None
    br_d = din("br128", [128, HC], F32) if has_bl else None
    gbT_d = din("gbT", [C, H], F32) if has_gb else None
    bcnn_d = din("bcnn128", [128, WE], F32) if has_bc else None
    mb1_d = din("mb1", [L1, 1], F32) if has_mb else None
    mb2_d = din("mb2", [L2, 1], F32) if has_mb else None
    mb3_d = din("mb3", [L3, 1], F32) if has_mb else None

    emb_out = nc.dram_tensor("emb_out", [BPC * BLK, WREAL], F32,
                             kind="ExternalOutput")
    pred_out = nc.dram_tensor("pred_out", [NPT, PT], F32,
                              kind="ExternalOutput")
    if debug:
        xl_out = nc.dram_tensor("xl_out", [NPAD, HC], F32,
                                kind="ExternalOutput")
        xr_out = nc.dram_tensor("xr_out", [NPAD, HC], F32,
                                kind="ExternalOutput")
        den_out = nc.dram_tensor("den_out", [BPC * H, BLK], F32,
                                 kind="ExternalOutput")
        e_out = nc.dram_tensor("e_out", [BPC * T * 128, H], F32,
                               kind="ExternalOutput")
        feat_out = nc.dram_tensor("feat_out", [128, WE], BF16,
                                  kind="ExternalOutput")
        gxl_out = nc.dram_tensor("gxl_out", [128, HC], F32,
                                 kind="ExternalOutput")
        zm_out = nc.dram_tensor("zm_out", [128, HC], F32,
                                kind="ExternalOutput")
        y_out = nc.dram_tensor("y_out", [128, HC], F32,
                               kind="ExternalOutput")
        ops_out = nc.dram_tensor("ops_out", [128, H * BLK], F32,
                                 kind="ExternalOutput")
        norm_out = nc.dram_tensor("norm_out", [128, H * BLK], F32,
                                  kind="ExternalOutput")

    ktiles_mic = _tiles(NM)   # 12
    ktiles_dis = _tiles(ND)   # 7
    jt1 = _tiles(L1)          # 11
    jt2 = _tiles(L2)          # 6
    jt3 = _tiles(L3)          # 4
    KT_FE = WE // 128         # 22

    with tile.TileContext(nc) as tc:
        # ---------------- persistent pools ----------------
        with tc.tile_pool(name="dram", bufs=1, space="DRAM") as dram, \
             tc.tile_pool(name="const", bufs=1) as const:
            xl_dram = dram.tile([NPAD, HC], F32)
            xr_dram = dram.tile([NPAD, HC], F32)
            ag_in = dram.tile([BPC * BLK, WE], BF16)
            emb_sh = dram.tile([NPAD, WE], BF16)
            rscr = dram.tile([H, BLK], F32)

            ident = const.tile([128, 128], BF16)
            make_identity(nc, ident)

            # ============ Phase A: hT = [WmicT @ micT | WdisT @ disT] ============
            with tc.tile_pool(name="projw", bufs=1) as projw, \
                 tc.tile_pool(name="proj", bufs=3) as proj, \
                 tc.tile_pool(name="proj_ps", bufs=2, space="PSUM") as proj_ps, \
                 tc.tile_pool(name="hT_pool", bufs=1) as hT_pool:
                hT = hT_pool.tile([128, NPAD], F32)
                wm_sb = projw.tile([128, len(ktiles_mic), C], F32)
                wd_sb = projw.tile([128, len(ktiles_dis), C], F32)
                for kt, (k0, kw) in enumerate(ktiles_mic):
                    nc.sync.dma_start(wm_sb[:kw, kt, :], wmic_d[k0:k0 + kw, :])
                for kt, (k0, kw) in enumerate(ktiles_dis):
                    nc.sync.dma_start(wd_sb[:kw, kt, :], wdis_d[k0:k0 + kw, :])

                for base, xT_d, w_sb, ktl, n_tot in (
                    (0, micT_d, wm_sb, ktiles_mic, NM),
                    (NM, disT_d, wd_sb, ktiles_dis, ND),
                ):
                    for n0, nw in _tiles(n_tot, 512):
                        ps = proj_ps.tile([128, 512], F32, tag="pps")
                        for kt, (k0, kw) in enumerate(ktl):
                            xt = proj.tile([128, 512], F32, tag="xt")
                            nc.sync.dma_start(xt[:kw, :nw],
                                              xT_d[k0:k0 + kw, n0:n0 + nw])
                            nc.tensor.matmul(ps[:, :nw], lhsT=w_sb[:kw, kt, :],
                                             rhs=xt[:kw, :nw],
                                             start=(kt == 0),
                                             stop=(kt == len(ktl) - 1))
                        nc.vector.tensor_copy(hT[:, base + n0:base + n0 + nw],
                                              ps[:, :nw])

                # ============ Phase B: xl/xr -> DRAM ============
                with tc.tile_pool(name="xw", bufs=1) as xw, \
                     tc.tile_pool(name="xbuf", bufs=3) as xbuf, \
                     tc.tile_pool(name="x_ps", bufs=4, space="PSUM") as x_ps:
                    wl_sb = xw.tile([128, HC], F32)
                    wr_sb = xw.tile([128, HC], F32)
                    nc.sync.dma_start(wl_sb[:], wl_d[:])
                    nc.sync.dma_start(wr_sb[:], wr_d[:])
                    if has_bl:
                        bl_sb = xw.tile([128, HC], F32)
                        br_sb = xw.tile([128, HC], F32)
                        nc.sync.dma_start(bl_sb[:], bl_d[:])
                        nc.sync.dma_start(br_sb[:], br_d[:])
                    for nt in range(NPAD // 128):
                        hTn = hT[:, nt * 128:(nt + 1) * 128]
                        for w_sb, b_name, x_d in ((wl_sb, "bl", xl_dram),
                                                  (wr_sb, "br", xr_dram)):
                            for ch in range(2):
                                cs = slice(ch * 512, (ch + 1) * 512)
                                ps = x_ps.tile([128, 512], F32, tag="xps")
                                nc.tensor.matmul(ps[:], lhsT=hTn,
                                                 rhs=w_sb[:, cs],
                                                 start=True, stop=True)
                                xs = xbuf.tile([128, 512], F32, tag="xs")
                                if has_bl:
                                    bsb = bl_sb if b_name == "bl" else br_sb
                                    nc.vector.tensor_add(xs[:], ps[:], bsb[:, cs])
                                else:
                                    nc.vector.tensor_copy(xs[:], ps[:])
                                nc.sync.dma_start(
                                    x_d[nt * 128:(nt + 1) * 128, cs], xs[:])
                                if debug:
                                    dbg_d = xl_out if x_d is xl_dram else xr_out
                                    nc.sync.dma_start(
                                        dbg_d[nt * 128:(nt + 1) * 128, cs],
                                        xs[:])

            # ============ Phase C: edge softmax + aggregation + CNN ============
            with tc.tile_pool(name="cw", bufs=1) as cw, \
                 tc.tile_pool(name="edge", bufs=2) as edge, \
                 tc.tile_pool(name="epost", bufs=2) as epost, \
                 tc.tile_pool(name="out_ps", bufs=2, space="PSUM") as out_ps, \
                 tc.tile_pool(name="den_ps", bufs=2, space="PSUM") as den_ps, \
                 tc.tile_pool(name="cnn_ps", bufs=2, space="PSUM") as cnn_ps:
                att_sb = cw.tile([128, HC], F32)
                nc.sync.dma_start(att_sb[:], att_d[:])
                wcnn_sb = cw.tile([128, H, WE], BF16)
                for kt in range(H):
                    nc.sync.dma_start(wcnn_sb[:, kt, :],
                                      wcnn_d[kt * 128:(kt + 1) * 128, :])
                eidx_sb = cw.tile([128, BPC * T, 2], I32)
                nc.sync.dma_start(eidx_sb[:], eidx_d[:])
                msel_sb = cw.tile([128, BPC * T, BLK], BF16)
                nc.sync.dma_start(msel_sb[:], msel_d[:])
                if has_gb:
                    gbT_sb = cw.tile([C, H], F32)
                    nc.sync.dma_start(gbT_sb[:], gbT_d[:])
                if has_bc:
                    bcnn_sb = cw.tile([128, WE], F32)
                    nc.sync.dma_start(bcnn_sb[:], bcnn_d[:])

                for blk in range(BPC):
                    # SBUF accumulators (PSUM accumulation across the t-loop
                    # is unsafe: Tile may reorder matmuls within a group)
                    acc_o = epost.tile([128, H * BLK], F32, tag="acc_o")
                    acc_d = epost.tile([H, BLK], F32, tag="acc_d")
                    for t in range(T):
                        ti = blk * T + t
                        gxl = edge.tile([128, HC], F32, tag="gxl", bufs=3)
                        gxr = edge.tile([128, HC], F32, tag="gxr", bufs=3)
                        nc.gpsimd.indirect_dma_start(
                            out=gxl[:], out_offset=None, in_=xl_dram[:],
                            in_offset=IndirectOffsetOnAxis(
                                ap=eidx_sb[:, ti, 0:1], axis=0))
                        nc.gpsimd.indirect_dma_start(
                            out=gxr[:], out_offset=None, in_=xr_dram[:],
                            in_offset=IndirectOffsetOnAxis(
                                ap=eidx_sb[:, ti, 1:2], axis=0))
                        zs = edge.tile([128, HC], F32, tag="zs")
                        nc.vector.tensor_add(zs[:], gxl[:], gxr[:])
                        z = edge.tile([128, HC], F32, tag="z")
                        # NB: the ACT Lrelu table is NEFF-global (one alpha);
                        # the MLP uses Lrelu(0.01), so the GAT slope rides on
                        # the separate Prelu table.
                        nc.scalar.activation(z[:], zs[:], AF.Prelu,
                                             alpha=GAT_SLOPE)
                        zm = edge.tile([128, HC], F32, tag="zm")
                        nc.vector.tensor_tensor(out=zm[:], in0=z[:],
                                                in1=att_sb[:], op=OP.mult)
                        if debug and ti == 0:
                            nc.sync.dma_start(gxl_out[:], gxl[:])
                            nc.sync.dma_start(zm_out[:], zm[:])
                        e = edge.tile([128, H], F32, tag="e")
                        nc.vector.tensor_reduce(
                            e[:], zm[:].rearrange("p (h c) -> p h c", c=C),
                            axis=mybir.AxisListType.X, op=OP.add)
                        ex = edge.tile([128, H], F32, tag="ex")
                        nc.scalar.activation(ex[:], e[:], AF.Exp)
                        if debug:
                            nc.sync.dma_start(
                                e_out[ti * 128:(ti + 1) * 128, :], e[:])
                        exb = edge.tile([128, H], BF16, tag="exb")
                        nc.vector.tensor_copy(exb[:], ex[:])
                        y = edge.tile([128, HC], BF16, tag="y")
                        nc.vector.tensor_tensor(
                            out=y[:].rearrange("p (h c) -> p h c", c=C),
                            in0=gxl[:].rearrange("p (h c) -> p h c", c=C),
                            in1=ex[:].to_broadcast([128, H, C]),
                            op=OP.mult)
                        if debug and ti == 0:
                            yd = edge.tile([128, HC], F32, tag="yd")
                            nc.vector.tensor_copy(yd[:], y[:])
                            nc.sync.dma_start(y_out[:], yd[:])
                        mst = msel_sb[:, ti, :]
                        ops = out_ps.tile([128, H * 128], F32, tag="ops")
                        dps = den_ps.tile([H, BLK], F32, tag="dps")
                        for ct in range(H):
                            nc.tensor.matmul(
                                ops[:, ct * 128:ct * 128 + BLK],
                                lhsT=y[:, ct * 128:(ct + 1) * 128],
                                rhs=mst, start=True, stop=True)
                        nc.tensor.matmul(dps[:], lhsT=exb[:], rhs=mst,
                                         start=True, stop=True)
                        ops_t = ops[:].rearrange(
                            "p (h c) -> p h c", c=128)[:, :, :BLK]
                        acc_v = acc_o[:].rearrange("p (h b) -> p h b", b=BLK)
                        if t == 0:
                            nc.vector.tensor_copy(acc_v, ops_t)
                            nc.vector.tensor_copy(acc_d[:], dps[:])
                        else:
                            nc.vector.tensor_add(acc_v, acc_v, ops_t)
                            nc.vector.tensor_add(acc_d[:], acc_d[:], dps[:])

                    # normalize + bias -> resT (bf16, [128, h*96])
                    if debug and blk == 0:
                        nc.sync.dma_start(ops_out[:], acc_o[:])
                    rT = epost.tile([H, BLK], F32, tag="rT")
                    nc.vector.reciprocal(rT[:], acc_d[:])
                    if debug:
                        nc.sync.dma_start(
                            den_out[blk * H:(blk + 1) * H, :], acc_d[:])
                    nc.sync.dma_start(rscr[:], rT[:])
                    rbc = epost.tile([128, H * BLK], F32, tag="rbc")
                    rscr_bcast = bass.AP(rscr[:].tensor, rscr[:].offset,
                                         [[0, 128], [BLK, H], [1, BLK]])
                    nc.sync.dma_start(
                        rbc[:].rearrange("p (h b) -> p h b", b=BLK), rscr_bcast)
                    normT = epost.tile([128, H * BLK], F32, tag="normT")
                    nc.vector.tensor_tensor(
                        out=normT[:], in0=acc_o[:], in1=rbc[:], op=OP.mult)
                    if debug and blk == 0:
                        nc.sync.dma_start(norm_out[:], normT[:])
                    resT = epost.tile([128, H * BLK], BF16, tag="resT")
                    if has_gb:
                        for h in range(H):
                            nc.scalar.activation(
                                resT[:, h * BLK:(h + 1) * BLK],
                                normT[:, h * BLK:(h + 1) * BLK],
                                AF.Identity, bias=gbT_sb[:, h:h + 1])
                    else:
                        nc.vector.tensor_copy(resT[:], normT[:])

                    # CNN: emb = relu(resT.T @ Wcnn + bcnn)
                    emb_f = epost.tile([BLK, WE], F32, tag="emb_f")
                    for c0, cwd in _tiles(WE, 512):
                        eps = cnn_ps.tile([BLK, 512], F32, tag="eps")
                        for kt in range(H):
                            nc.tensor.matmul(
                                eps[:, :cwd],
                                lhsT=resT[:, kt * BLK:(kt + 1) * BLK],
                                rhs=wcnn_sb[:, kt, c0:c0 + cwd],
                                start=(kt == 0), stop=(kt == H - 1))
                        if has_bc:
                            tmp = epost.tile([BLK, 512], F32, tag="ctmp")
                            nc.vector.tensor_add(tmp[:, :cwd], eps[:, :cwd],
                                                 bcnn_sb[:BLK, c0:c0 + cwd])
                            nc.scalar.activation(emb_f[:, c0:c0 + cwd],
                                                 tmp[:, :cwd], AF.Relu)
                        else:
                            nc.scalar.activation(emb_f[:, c0:c0 + cwd],
                                                 eps[:, :cwd], AF.Relu)
                    embb = epost.tile([BLK, WE], BF16, tag="embb")
                    nc.vector.tensor_copy(embb[:], emb_f[:])
                    nc.sync.dma_start(
                        emb_out[blk * BLK:(blk + 1) * BLK, :],
                        emb_f[:, :WREAL])
                    nc.sync.dma_start(
                        ag_in[blk * BLK:(blk + 1) * BLK, :], embb[:])

            # ============ AllGather ============
            nc.gpsimd.collective_compute(
                "AllGather", OP.bypass,
                replica_groups=[list(range(N_CORES))],
                ins=[ag_in[:].opt()], outs=[emb_sh[:].opt()])

            # ============ Phase D: pair gather + MLP ============
            with tc.tile_pool(name="mwp", bufs=1) as mwp, \
                 tc.tile_pool(name="mlp", bufs=2) as mlp, \
                 tc.tile_pool(name="mg", bufs=2) as mg, \
                 tc.tile_pool(name="t_ps", bufs=2, space="PSUM") as t_ps, \
                 tc.tile_pool(name="m_ps", bufs=1, space="PSUM") as m_ps:
                mw1_sb = mwp.tile([128, KT_FE, L1], BF16)
                for kt in range(KT_FE):
                    nc.sync.dma_start(mw1_sb[:, kt, :],
                                      mw1_d[kt * 128:(kt + 1) * 128, :])
                mw2_sb = mwp.tile([128, len(jt1), L2], BF16)
                for kt, (k0, kw) in enumerate(jt1):
                    nc.sync.dma_start(mw2_sb[:kw, kt, :], mw2_d[k0:k0 + kw, :])
                mw3_sb = mwp.tile([128, len(jt2), L3], BF16)
                for kt, (k0, kw) in enumerate(jt2):
                    nc.sync.dma_start(mw3_sb[:kw, kt, :], mw3_d[k0:k0 + kw, :])
                mw4_sb = mwp.tile([128, len(jt3), 1], BF16)
                for kt, (k0, kw) in enumerate(jt3):
                    nc.sync.dma_start(mw4_sb[:kw, kt, :], mw4_d[k0:k0 + kw, :])
                pidx_sb = mwp.tile([128, NPT, 4, 2], I32)
                nc.sync.dma_start(pidx_sb[:], pidx_d[:])
                if has_mb:
                    mb1_sb = mwp.tile([L1, 1], F32)
                    mb2_sb = mwp.tile([L2, 1], F32)
                    mb3_sb = mwp.tile([L3, 1], F32)
                    nc.sync.dma_start(mb1_sb[:], mb1_d[:])
                    nc.sync.dma_start(mb2_sb[:], mb2_d[:])
                    nc.sync.dma_start(mb3_sb[:], mb3_d[:])

                for pt in range(NPT):
                    fT = mlp.tile([128, KT_FE, PT], BF16, tag="fT")
                    for sub in range(4):
                        gm = mg.tile([128, WE], BF16, tag="gm")
                        gd = mg.tile([128, WE], BF16, tag="gd")
                        nc.gpsimd.indirect_dma_start(
                            out=gm[:], out_offset=None, in_=emb_sh[:],
                            in_offset=IndirectOffsetOnAxis(
                                ap=pidx_sb[:, pt, sub, 0:1], axis=0))
                        nc.gpsimd.indirect_dma_start(
                            out=gd[:], out_offset=None, in_=emb_sh[:],
                            in_offset=IndirectOffsetOnAxis(
                                ap=pidx_sb[:, pt, sub, 1:2], axis=0))
                        ft = mg.tile([128, WE], BF16, tag="ft")
                        nc.vector.tensor_tensor(out=ft[:], in0=gm[:],
                                                in1=gd[:], op=OP.mult)
                        if debug and pt == 0 and sub == 0:
                            nc.sync.dma_start(feat_out[:], ft[:])
                        for kt in range(KT_FE):
                            tp = t_ps.tile([128, 128], BF16, tag="tp")
                            nc.tensor.transpose(
                                tp[:], ft[:, kt * 128:(kt + 1) * 128], ident[:])
                            nc.vector.tensor_copy(
                                fT[:, kt, sub * 128:(sub + 1) * 128], tp[:])

                    h1 = mlp.tile([128, len(jt1), PT], BF16, tag="h1", bufs=1)
                    for jt, (j0, jw) in enumerate(jt1):
                        ps = m_ps.tile([128, PT], F32, tag="ps1", bufs=2)
                        for kt in range(KT_FE):
                            nc.tensor.matmul(ps[:jw, :],
                                             lhsT=mw1_sb[:, kt, j0:j0 + jw],
                                             rhs=fT[:, kt, :],
                                             start=(kt == 0),
                                             stop=(kt == KT_FE - 1))
                        nc.scalar.activation(
                            h1[:jw, jt, :], ps[:jw, :], AF.Lrelu,
                            bias=(mb1_sb[j0:j0 + jw, :] if has_mb else 0.0),
                            alpha=MLP_SLOPE)
                    h2 = mlp.tile([128, len(jt2), PT], BF16, tag="h2", bufs=1)
                    for jt, (j0, jw) in enumerate(jt2):
                        ps = m_ps.tile([128, PT], F32, tag="ps2")
                        for kt, (k0, kw) in enumerate(jt1):
                            nc.tensor.matmul(ps[:jw, :],
                                             lhsT=mw2_sb[:kw, kt, j0:j0 + jw],
                                             rhs=h1[:kw, kt, :],
                                             start=(kt == 0),
                                             stop=(kt == len(jt1) - 1))
                        nc.scalar.activation(
                            h2[:jw, jt, :], ps[:jw, :], AF.Lrelu,
                            bias=(mb2_sb[j0:j0 + jw, :] if has_mb else 0.0),
                            alpha=MLP_SLOPE)
                    h3 = mlp.tile([128, len(jt3), PT], BF16, tag="h3", bufs=1)
                    for jt, (j0, jw) in enumerate(jt3):
                        ps = m_ps.tile([128, PT], F32, tag="ps3")
                        for kt, (k0, kw) in enumerate(jt2):
                            nc.tensor.matmul(ps[:jw, :],
                                             lhsT=mw3_sb[:kw, kt, j0:j0 + jw],
                                             rhs=h2[:kw, kt, :],
                                             start=(kt == 0),
                                             stop=(kt == len(jt2) - 1))
                        nc.scalar.activation(
                            h3[:jw, jt, :], ps[:jw, :], AF.Lrelu,
                            bias=(mb3_sb[j0:j0 + jw, :] if has_mb else 0.0),
                            alpha=MLP_SLOPE)
                    ps4 = m_ps.tile([1, PT], F32, tag="ps4")
                    for kt, (k0, kw) in enumerate(jt3):
                        nc.tensor.matmul(ps4[:], lhsT=mw4_sb[:kw, kt, :],
                                         rhs=h3[:kw, kt, :],
                                         start=(kt == 0),
                                         stop=(kt == len(jt3) - 1))
                    sg = mg.tile([1, PT], F32, tag="sg")
                    nc.scalar.activation(sg[:], ps4[:], AF.Sigmoid)
                    nc.sync.dma_start(pred_out[pt:pt + 1, :], sg[:])

    nc.compile()
    return nc


_CACHE = {}


def _in_maps(d):
    common = {
        "micT": d["micT"], "disT": d["disT"],
        "W_mic": d["W_mic"], "W_dis": d["W_dis"],
        "Wl": d["Wl"], "Wr": d["Wr"], "att128": d["att128"],
        "Wcnn": d["Wcnn"], "mw1": d["mw1"], "mw2": d["mw2"],
        "mw3": d["mw3"], "mw4": d["mw4"],
    }
    if d["has_bl"]:
        common["bl128"] = d["bl128"]
        common["br128"] = d["br128"]
    if d["has_gb"]:
        common["gbT"] = d["gbT"]
    if d["has_bc"]:
        common["bcnn128"] = d["bcnn128"]
    if d["has_mb"]:
        common["mb1"], common["mb2"], common["mb3"] = d["mb1"], d["mb2"], d["mb3"]
    in_maps = []
    for c in range(N_CORES):
        m = dict(common)
        m["eidx"] = d["eidx_pc"][c]
        m["msel"] = d["msel_pc"][c]
        m["pidx"] = d["pidx_pc"][c]
        in_maps.append(m)
    return in_maps


def _run(inputs, **run_kwargs):
    d = _prep_host(inputs)
    key = (d["T"], d["has_bl"], d["has_gb"], d["has_bc"], d["has_mb"])
    if key not in _CACHE:
        _CACHE[key] = _build_program(*key)
    nc = _CACHE[key]
    res = run_bass_kernel_spmd(nc, _in_maps(d), list(range(N_CORES)),
                               **run_kwargs)
    cnn_outputs = np.concatenate(
        [res.results[c]["emb_out"] for c in range(N_CORES)], axis=0)[:NN]
    pred = np.concatenate(
        [res.results[c]["pred_out"].reshape(PPAD, 1)[:PPC]
         for c in range(N_CORES)], axis=0)
    out = (pred.astype(np.float32), d["labels"], cnn_outputs.astype(np.float32))
    return out, res


def kernel(**inputs):
    out, _ = _run(inputs)
    return out
